# revision 44
# baseline (speedup 1.0000x reference)
"""Trainium2 Bass kernel for GQA attention (B=8, S=1024, H=2048, 32 Q / 8 KV heads, D=64).

Data-parallel over batch: one batch element per NeuronCore, weights
replicated, zero collectives. Per-core pipeline (all matmuls float32r for
projections/O-proj, bfloat16 for the attention inner loops):

  1. PE-transpose hidden -> hiddenT [H, S] (f32r, identity matmul).
  2. Q/K/V projections off hiddenT with double-buffered streamed weight
     chunks; q/k emerge in [d, s] layout, RoPE applied via partition-shift
     SBUF DMAs + DVE mul/add, then q spills to DRAM (bf16) and k is
     duplicated into both 64-partition slots of kT. v is stored natural
     [s, d] with a ones column appended (v_aug).
  3. Per head, per 128-row key tile jt: scoresT[j, i] = kT^T q (K=64 bf16
     matmuls, causal i>=128*jt half-open range only), causal diagonal mask
     added via identity-matmul of a precomputed mask tile, one merged exp on
     ScalarE per jt, then the PV matmul with v_aug (M=65) accumulates both
     the attention output and the softmax denominator (row 64).
  4. Per-head normalization: denominator row -> partition 0 via DMA,
     reciprocal_approx_fast, bf16 cast, ones-column matmul broadcasts it
     across 64 partitions in PSUM, fused DVE multiply, DMA into attT.
  5. O-projection (f32r) with streamed Wo chunks.

Timing feedback comes from the HW-validated instruction cost model
(no-exec CoreSim schedule): ~609 us/core. Relative error ~3.4e-3.
"""

import contextlib

import numpy as np

import concourse.bass as bass
import concourse.tile as tile
from concourse import bacc, mybir
from concourse.bass_utils import run_bass_kernel_spmd

B, S, H = 8, 1024, 2048
NQ, NKV, D = 32, 8, 64
F32 = mybir.dt.float32
F32R = mybir.dt.float32r
BF16 = mybir.dt.bfloat16
NEG = -1.0e30
AF = mybir.ActivationFunctionType


def _tables():
    inv = 1.0 / (10000.0 ** (np.arange(0, D, 2, dtype=np.float64) / D))  # [32]
    fr = np.arange(S, dtype=np.float64)[:, None] * inv[None, :]  # [S, 32]
    cos = np.cos(fr).T  # [32, S]
    sin = np.sin(fr).T
    cosT = np.concatenate([cos, cos], 0)  # [64, S]
    sgnT = np.concatenate([-sin, sin], 0)  # [64, S]
    cos128 = np.concatenate([cosT, cosT], 0).astype(np.float32)  # [128, S]
    sgn128 = np.concatenate([sgnT, sgnT], 0).astype(np.float32)
    p = np.arange(128)[:, None]
    c = np.arange(512)[None, :]
    masks = np.concatenate(
        [np.where(p <= c - 128 * m, 0.0, NEG) for m in range(4)], axis=0
    ).astype(np.float32)  # [512, 512]
    ident = np.eye(128, dtype=np.float32)
    return cos128, sgn128, masks, ident


def _rope(nc, rp, ps, cos_sl, sgn_sl):
    """psum [128,512] (raw qT/kT tile) -> SBUF tile with RoPE applied."""
    raw = rp.tile([128, 512], F32, name="rope_raw", tag="rope_raw")
    nc.scalar.copy(raw[:], ps[:])
    sh = rp.tile([128, 512], F32, name="rope_sh", tag="rope_sh")
    for a in range(4):  # partition quarter a reads quarter a^1  (p -> p xor 32)
        sc = (a ^ 1) * 32
        eng = nc.sync if a % 2 == 0 else nc.gpsimd
        eng.dma_start(out=sh[a * 32 : (a + 1) * 32, :], in_=raw[sc : sc + 32, :])
    tmp = rp.tile([128, 512], F32, name="rope_tmp", tag="rope_tmp")
    nc.vector.tensor_mul(tmp[:], raw[:], cos_sl)
    rot = rp.tile([128, 512], F32, name="rope_rot", tag="rope_rot")
    nc.gpsimd.tensor_mul(rot[:], sh[:], sgn_sl)
    fin = rp.tile([128, 512], BF16, name="rope_fin", tag="rope_fin")
    nc.vector.tensor_add(fin[:], tmp[:], rot[:])
    return fin


def _body(nc, tc, ctx, hid, wq, wk, wv, wo, cosd, sgnd, maskd, identd, onesd, onesrd, outd, qt_dram, dbg=None):
    # ---- constants (live whole body) ----
    cpool = ctx.enter_context(tc.tile_pool(name="const", bufs=1))
    ident_r = cpool.tile([128, 128], F32R, name="ident_r", tag="ident_r")
    nc.sync.dma_start(ident_r[:], identd[:].bitcast(F32R))

    with contextlib.ExitStack() as proj_ctx:
        tabp = proj_ctx.enter_context(tc.tile_pool(name="ropetab", bufs=1))
        cos128 = tabp.tile([128, S], F32, name="cos", tag="cos")
        nc.scalar.dma_start(cos128[:], cosd[:])
        sgn128 = tabp.tile([128, S], F32, name="sgn", tag="sgn")
        nc.scalar.dma_start(sgn128[:], sgnd[:])
        # shared weight-chunk pool: wv/wk/wq all stream [128, 8*512] chunks
        wbufp = proj_ctx.enter_context(tc.tile_pool(name="wbuf", bufs=4))
        # hT lives phases 1-4; va/kT live phases 2-5 (opened here, closed later)
        hTpool = proj_ctx.enter_context(tc.tile_pool(name="hTp", bufs=1))
        hT = [hTpool.tile([128, S], F32R, name=f"hT{c}", tag=f"hT{c}") for c in range(16)]

        attn_ctx = contextlib.ExitStack()
        vapool = attn_ctx.enter_context(tc.tile_pool(name="vap", bufs=1, side="right"))
        va = [
            vapool.tile([128, 8 * 65], BF16, name=f"va{s}", tag=f"va{s}")
            for s in range(8)
        ]
        kpool = attn_ctx.enter_context(tc.tile_pool(name="kTp", bufs=1, side="right"))
        kT = kpool.tile([128, 8 * S], BF16, name="kT", tag="kT")

        # ================= Phase 1: transpose hidden =================
        with tc.tile_pool(name="hidnat", bufs=5) as hp, tc.tile_pool(
            name="tpsum", bufs=6, space="PSUM"
        ) as tp:
            for half in range(2):
                hid_nat = []
                for tt in range(4):
                    t = half * 4 + tt
                    ht = hp.tile([128, H], F32R, name="hidnat", tag="hidnat")
                    nc.sync.dma_start(ht[:], hid[t * 128 : (t + 1) * 128, :].bitcast(F32R))
                    hid_nat.append(ht)
                for c in range(16):
                    ps = tp.tile([128, 512], F32R, name="tp", tag="tp")
                    for tt in range(4):
                        nc.tensor.transpose(
                            ps[:, tt * 128 : (tt + 1) * 128],
                            hid_nat[tt][:, c * 128 : (c + 1) * 128],
                            ident_r[:],
                        )
                    nc.scalar.copy(hT[c][:, half * 512 : (half + 1) * 512], ps[:])

        # ================= Phase 2: V projection (+ ones col) =========
        with tc.tile_pool(name="vpsum", bufs=8, space="PSUM") as vps:
            wv_t = []
            for c in range(2):
                wvm = wbufp.tile([128, 8 * 512], F32R, name="wvm", tag="wchunk")
                nc.sync.dma_start(
                    wvm.rearrange("p (t f) -> p t f", t=8),
                    wv.rearrange("(t p) f -> p t f", p=128)[:, c * 8 : c * 8 + 8].bitcast(F32R),
                )
                wv_t += [wvm[:, h * 512 : (h + 1) * 512] for h in range(8)]
            for st in range(8):
                ps = vps.tile([128, 512], F32, name="vp", tag="vp")
                for h in range(16):
                    nc.tensor.matmul(
                        ps[:],
                        hT[h][:, st * 128 : (st + 1) * 128],
                        wv_t[h],
                        start=(h == 0),
                        stop=(h == 15),
                    )
                va3 = va[st].rearrange("p (g c) -> p g c", c=65)
                nc.scalar.copy(
                    va3[:, :, 0:64], ps[:].rearrange("p (g c) -> p g c", c=64)
                )
                nc.gpsimd.dma_start(out=va3[:, :, 64:65], in_=onesd[st * 128 : (st + 1) * 128, :].rearrange("p (g c) -> p g c", c=1))

        # ============ Phase 3: K projection + RoPE + slot duplication ==
        with tc.tile_pool(
            name="kpsum", bufs=8, space="PSUM"
        ) as kps, tc.tile_pool(name="krope", bufs=4) as krp:
            wk_t = []
            for c in range(2):
                wkm = wbufp.tile([128, 8 * 512], F32R, name="wkm", tag="wchunk")
                nc.sync.dma_start(
                    wkm.rearrange("p (t f) -> p t f", t=8),
                    wk.rearrange("(t p) f -> p t f", p=128)[:, c * 8 : c * 8 + 8].bitcast(F32R),
                )
                wk_t += [wkm[:, h * 512 : (h + 1) * 512] for h in range(8)]
            for ft in range(4):
                for ih in range(2):
                    ps = kps.tile([128, 512], F32, name="kp", tag="kp")
                    for h in range(16):
                        nc.tensor.matmul(
                            ps[:],
                            wk_t[h][:, ft * 128 : (ft + 1) * 128],
                            hT[h][:, ih * 512 : (ih + 1) * 512],
                            start=(h == 0),
                            stop=(h == 15),
                        )
                    sl = slice(ih * 512, (ih + 1) * 512)
                    kfin = _rope(nc, krp, ps, cos128[:, sl], sgn128[:, sl])
                    b0, b1 = 2 * ft, 2 * ft + 1
                    o0 = b0 * S + ih * 512
                    o1 = b1 * S + ih * 512
                    nc.scalar.dma_start(kT[0:64, o0 : o0 + 512], kfin[0:64, :])
                    nc.gpsimd.dma_start(out=kT[64:128, o0 : o0 + 512], in_=kfin[0:64, :])
                    nc.scalar.dma_start(kT[64:128, o1 : o1 + 512], kfin[64:128, :])
                    nc.gpsimd.dma_start(out=kT[0:64, o1 : o1 + 512], in_=kfin[64:128, :])

        # ========= Phase 4: Q projection + RoPE -> DRAM spill ==========
        with tc.tile_pool(
            name="qpsum", bufs=8, space="PSUM"
        ) as qps, tc.tile_pool(name="qrope", bufs=4) as qrp:
            for wh in range(4):
                wq_t = []
                for c in range(2):
                    wqm = wbufp.tile([128, 8 * 512], F32R, name="wqm", tag="wchunk")
                    nc.sync.dma_start(
                        wqm.rearrange("p (t f) -> p t f", t=8),
                        wq.rearrange("(t p) f -> p t f", p=128)[
                            :, c * 8 : c * 8 + 8, wh * 512 : (wh + 1) * 512
                        ].bitcast(F32R),
                    )
                    wq_t += [wqm[:, h * 512 : (h + 1) * 512] for h in range(8)]
                for ftl in range(4):
                    ft = wh * 4 + ftl
                    for ih in range(2):
                        ps = qps.tile([128, 512], F32, name="qp", tag="qp")
                        for h in range(16):
                            nc.tensor.matmul(
                                ps[:],
                                wq_t[h][:, ftl * 128 : (ftl + 1) * 128],
                                hT[h][:, ih * 512 : (ih + 1) * 512],
                                start=(h == 0),
                                stop=(h == 15),
                            )
                        sl = slice(ih * 512, (ih + 1) * 512)
                        qfin = _rope(nc, qrp, ps, cos128[:, sl], sgn128[:, sl])
                        off = ft * S + ih * 512
                        nc.scalar.dma_start(qt_dram[:, off : off + 512], qfin[:])

    # hT freed here; attn_ctx (va, kT) still open
    # ================= Phase 5: attention =================
    mkp = ctx.enter_context(tc.tile_pool(name="masks", bufs=1))
    mask_b = mkp.tile([128, 128], BF16, name="mask_b", tag="mask_b")
    nc.gpsimd.dma_start(out=mask_b[:], in_=maskd[0:128, 0:128])
    ident_b = mkp.tile([128, 128], BF16, name="ident_b", tag="ident_b")
    nc.gpsimd.dma_start(out=ident_b[:], in_=identd[:])
    ones_r = mkp.tile([1, 64], BF16, name="ones_r", tag="ones_r")
    nc.gpsimd.dma_start(out=ones_r[:], in_=onesrd[:])
    wo0 = mkp.tile([128, 8 * 512], F32R, name="wo0", tag="wo0")
    nc.sync.dma_start(
        wo0.rearrange("p (t f) -> p t f", t=8),
        wo.rearrange("(t p) f -> p t f", p=128)[:, 0:8, 0:512].bitcast(F32R),
    )
    apool = ctx.enter_context(tc.tile_pool(name="attTp", bufs=1))
    attT = apool.tile([128, 16 * S], F32R, name="attT", tag="attT")

    with tc.tile_pool(name="qst", bufs=4) as qsp, tc.tile_pool(
        name="scpsum", bufs=2, space="PSUM"
    ) as scp, tc.tile_pool(name="pvpsum", bufs=1, space="PSUM") as pvp, tc.tile_pool(
        name="expT", bufs=5
    ) as exp_p, tc.tile_pool(name="pvsb", bufs=4) as pvsbp, tc.tile_pool(
        name="dbps", bufs=1, space="PSUM"
    ) as dbp, tc.tile_pool(name="rrowp", bufs=3) as rrp:
        for bq in range(16):
            qs = qsp.tile([128, 1024], BF16, name="qs", tag="qs")
            nc.sync.dma_start(qs[:], qt_dram[:, bq * S : bq * S + 1024])
            for hs in range(2):
                h = 2 * bq + hs
                g = h // 4
                slot = 64 * hs
                pv = pvp.tile([65, 1024], F32, name="pv", tag="pv")
                pvs = pvsbp.tile([65, 1024], F32R, name="pvs", tag="pvs")
                for jt in range(8):
                    lo = jt * 128
                    sc = scp.tile([128, 1024], F32, name="sc", tag="sc")
                    kap = kT[slot : slot + 64, g * S + lo : g * S + lo + 128]
                    qap = qs[slot : slot + 64, :]
                    vab = va[jt].rearrange("p (g c) -> p g c", c=65)[:, g, :]
                    if jt < 4:
                        nc.tensor.matmul(
                            sc[:, lo:512], kap, qap[:, lo:512],
                            start=True, stop=False, skip_group_check=True,
                        )
                        nc.tensor.matmul(
                            sc[:, 512:1024], kap, qap[:, 512:1024],
                            start=True, stop=True, skip_group_check=True,
                        )
                        nc.tensor.matmul(
                            sc[:, lo : lo + 128], ident_b[:], mask_b[:],
                            start=False, stop=True, skip_group_check=True,
                        )
                        ex = exp_p.tile([128, 1024], BF16, name="ex", tag="ex")
                        nc.scalar.activation(ex[:, lo:1024], sc[:, lo:1024], AF.Exp, scale=0.125)
                        nc.tensor.matmul(
                            pv[:, lo:512], vab, ex[:, lo:512],
                            start=(jt == 0), stop=(jt == 3), skip_group_check=True,
                        )
                        nc.tensor.matmul(
                            pv[:, 512:1024], vab, ex[:, 512:1024],
                            start=(jt == 0), stop=(jt == 7), skip_group_check=True,
                        )
                    else:
                        nc.tensor.matmul(
                            sc[:, lo:1024], kap, qap[:, lo:1024],
                            start=True, stop=False, skip_group_check=True,
                        )
                        nc.tensor.matmul(
                            sc[:, lo : lo + 128], ident_b[:], mask_b[:],
                            start=False, stop=True, skip_group_check=True,
                        )
                        ex = exp_p.tile([128, 1024], BF16, name="ex", tag="ex")
                        nc.scalar.activation(ex[:, lo:1024], sc[:, lo:1024], AF.Exp, scale=0.125)
                        nc.tensor.matmul(
                            pv[:, lo:1024], vab, ex[:, lo:1024],
                            start=False, stop=(jt == 7), skip_group_check=True,
                        )
                nc.vector.tensor_copy(pvs[:], pv[:])
                dstg = rrp.tile([1, 1024], F32, name="dstg", tag="dstg")
                nc.sync.dma_start(dstg[:], pvs[64:65, :].bitcast(F32))
                rrow = rrp.tile([1, 1024], F32, name="rrow", tag="rrow")
                nc.vector.reciprocal_approx_fast(rrow[:], dstg[:])
                rrb = rrp.tile([1, 1024], BF16, name="rrb", tag="rrb")
                nc.vector.tensor_copy(rrb[:], rrow[:])
                db = dbp.tile([64, 1024], F32, name="db", tag="db")
                for ih in range(2):
                    nc.tensor.matmul(
                        db[:, ih * 512 : ih * 512 + 512],
                        ones_r[:],
                        rrb[0:1, ih * 512 : ih * 512 + 512],
                        start=True,
                        stop=True,
                    )
                pvn = pvsbp.tile([64, 1024], F32R, name="pvn", tag="pvn")
                nc.vector.tensor_mul(pvn[:], pvs[0:64, :], db[:].bitcast(F32R))
                nc.sync.dma_start(
                    attT[slot : slot + 64, bq * S : bq * S + 1024], pvn[:]
                )
                if dbg is not None and h == 0:
                    nc.sync.dma_start(dbg["pvs0"][:], pvs[:].bitcast(F32))
                    dbsb = pvsbp.tile([64, 1024], F32, name="dbsb", tag="dbsb")
                    nc.vector.tensor_copy(dbsb[:], db[:])
                    nc.sync.dma_start(dbg["db0"][:], dbsb[:])

    attn_ctx.close()  # free va, kT

    if dbg is not None:
        nc.sync.dma_start(dbg["attT"][:], attT[:].bitcast(F32))

    # ================= O projection ================
    with tc.tile_pool(name="wo", bufs=4) as wop, tc.tile_pool(
        name="opsum", bufs=4, space="PSUM"
    ) as ops, tc.tile_pool(name="osb", bufs=6) as osbp:
        for ho in range(4):
            woc = []
            for c in range(2):
                if ho == 0 and c == 0:
                    woc += [wo0[:, ft * 512 : (ft + 1) * 512] for ft in range(8)]
                    continue
                wom = wop.tile([128, 8 * 512], F32R, name="wom", tag="wom")
                nc.sync.dma_start(
                    wom.rearrange("p (t f) -> p t f", t=8),
                    wo.rearrange("(t p) f -> p t f", p=128)[
                        :, c * 8 : c * 8 + 8, ho * 512 : (ho + 1) * 512
                    ].bitcast(F32R),
                )
                woc += [wom[:, ft * 512 : (ft + 1) * 512] for ft in range(8)]
            for st in range(8):
                ps = ops.tile([128, 512], F32, name="op", tag="op")
                for ft in range(16):
                    nc.tensor.matmul(
                        ps[:],
                        attT[:, ft * S + st * 128 : ft * S + st * 128 + 128],
                        woc[ft],
                        start=(ft == 0),
                        stop=(ft == 15),
                    )
                ob = osbp.tile([128, 512], F32, name="ob", tag="ob")
                nc.scalar.copy(ob[:], ps[:])
                nc.gpsimd.dma_start(
                    out=outd[st * 128 : (st + 1) * 128, ho * 512 : (ho + 1) * 512],
                    in_=ob[:],
                )


def _build(niter=1, debug=False):
    nc = bacc.Bacc(None, target_bir_lowering=False)
    hid = nc.declare_dram_parameter("hidden_states", [S, H], F32, isOutput=False)
    wq = nc.declare_dram_parameter("Wq", [H, NQ * D], F32, isOutput=False)
    wk = nc.declare_dram_parameter("Wk", [H, NKV * D], F32, isOutput=False)
    wv = nc.declare_dram_parameter("Wv", [H, NKV * D], F32, isOutput=False)
    wo = nc.declare_dram_parameter("Wo", [NQ * D, H], F32, isOutput=False)
    cosd = nc.declare_dram_parameter("rope_cos", [128, S], F32, isOutput=False)
    sgnd = nc.declare_dram_parameter("rope_sgnsin", [128, S], F32, isOutput=False)
    maskd = nc.declare_dram_parameter("causal_masks", [512, 512], F32, isOutput=False)
    identd = nc.declare_dram_parameter("ident", [128, 128], F32, isOutput=False)
    onesd = nc.declare_dram_parameter("ones_col", [S, 8], F32, isOutput=False)
    onesrd = nc.declare_dram_parameter("ones_row", [1, 64], F32, isOutput=False)
    outd = nc.declare_dram_parameter("out", [S, H], F32, isOutput=True)
    dbg = None
    if debug:
        dbg = {
            "attT": nc.declare_dram_parameter("dbg_attT", [128, 16 * S], F32, isOutput=True),
            "pvs0": nc.declare_dram_parameter("dbg_pvs0", [65, 1024], F32, isOutput=True),
            "db0": nc.declare_dram_parameter("dbg_db0", [64, 1024], F32, isOutput=True),
        }
    qt_dram = nc.dram_tensor("qt_spill", [128, 16 * S], BF16)

    with tile.TileContext(nc) as tc:
        for _ in range(niter):
            with contextlib.ExitStack() as ctx:
                _body(
                    nc, tc, ctx, hid, wq, wk, wv, wo, cosd, sgnd, maskd, identd,
                    onesd, onesrd, outd, qt_dram, dbg,
                )
    nc.compile()
    return nc


_CACHE = {}


def _get_nc(niter=1):
    if niter not in _CACHE:
        _CACHE[niter] = _build(niter)
    return _CACHE[niter]


def _in_maps(inputs):
    cos128, sgn128, masks, ident = _tables()
    hidden = np.ascontiguousarray(inputs["hidden_states"], dtype=np.float32)
    base = {
        "Wq": np.ascontiguousarray(inputs["Wq"], dtype=np.float32),
        "Wk": np.ascontiguousarray(inputs["Wk"], dtype=np.float32),
        "Wv": np.ascontiguousarray(inputs["Wv"], dtype=np.float32),
        "Wo": np.ascontiguousarray(inputs["Wo"], dtype=np.float32),
        "rope_cos": cos128,
        "rope_sgnsin": sgn128,
        "causal_masks": masks,
        "ident": ident,
        "ones_col": np.ones((S, 8), np.float32),
        "ones_row": np.ones((1, 64), np.float32),
    }
    return [dict(base, hidden_states=hidden[i]) for i in range(B)]


def kernel(**inputs):
    nc = _get_nc(1)
    res = run_bass_kernel_spmd(nc, _in_maps(inputs), core_ids=list(range(8)))
    return np.stack([res.results[i]["out"] for i in range(B)]).astype(np.float32)


# revision 47
# speedup vs baseline: 1.0236x; 1.0236x over previous
"""Trainium2 Bass kernel for GQA attention (B=8, S=1024, H=2048, 32 Q / 8 KV heads, D=64).

Data-parallel over batch: one batch element per NeuronCore, weights
replicated, zero collectives. Per-core pipeline (all matmuls float32r for
projections/O-proj, bfloat16 for the attention inner loops):

  1. PE-transpose hidden -> hiddenT [H, S] (f32r, identity matmul).
  2. Q/K/V projections off hiddenT with double-buffered streamed weight
     chunks; q/k emerge in [d, s] layout, RoPE applied via partition-shift
     SBUF DMAs + DVE mul/add, then q spills to DRAM (bf16) and k is
     duplicated into both 64-partition slots of kT. v is stored natural
     [s, d] with a ones column appended (v_aug).
  3. Per head, per 128-row key tile jt: scoresT[j, i] = kT^T q (K=64 bf16
     matmuls, causal i>=128*jt half-open range only), causal diagonal mask
     added via identity-matmul of a precomputed mask tile, one merged exp on
     ScalarE per jt, then the PV matmul with v_aug (M=65) accumulates both
     the attention output and the softmax denominator (row 64).
  4. Per-head normalization: denominator row -> partition 0 via DMA,
     reciprocal_approx_fast, bf16 cast, ones-column matmul broadcasts it
     across 64 partitions in PSUM, fused DVE multiply, DMA into attT.
  5. O-projection (f32r) with streamed Wo chunks.

Timing feedback comes from the HW-validated instruction cost model
(no-exec CoreSim schedule): ~609 us/core. Relative error ~3.4e-3.
"""

import contextlib

import numpy as np

import concourse.bass as bass
import concourse.tile as tile
from concourse import bacc, mybir
from concourse.bass_utils import run_bass_kernel_spmd

B, S, H = 8, 1024, 2048
NQ, NKV, D = 32, 8, 64
F32 = mybir.dt.float32
F32R = mybir.dt.float32r
BF16 = mybir.dt.bfloat16
NEG = -1.0e30
AF = mybir.ActivationFunctionType


def _tables():
    inv = 1.0 / (10000.0 ** (np.arange(0, D, 2, dtype=np.float64) / D))  # [32]
    fr = np.arange(S, dtype=np.float64)[:, None] * inv[None, :]  # [S, 32]
    cos = np.cos(fr).T  # [32, S]
    sin = np.sin(fr).T
    cosT = np.concatenate([cos, cos], 0)  # [64, S]
    sgnT = np.concatenate([-sin, sin], 0)  # [64, S]
    cos128 = np.concatenate([cosT, cosT], 0).astype(np.float32)  # [128, S]
    sgn128 = np.concatenate([sgnT, sgnT], 0).astype(np.float32)
    p = np.arange(128)[:, None]
    c = np.arange(512)[None, :]
    masks = np.concatenate(
        [np.where(p <= c - 128 * m, 0.0, NEG) for m in range(4)], axis=0
    ).astype(np.float32)  # [512, 512]
    ident = np.eye(128, dtype=np.float32)
    return cos128, sgn128, masks, ident


def _rope(nc, rp, ps, cos_sl, sgn_sl):
    """psum [128,512] (raw qT/kT tile) -> SBUF tile with RoPE applied."""
    raw = rp.tile([128, 512], F32, name="rope_raw", tag="rope_raw")
    nc.scalar.copy(raw[:], ps[:])
    sh = rp.tile([128, 512], F32, name="rope_sh", tag="rope_sh")
    for a in range(4):  # partition quarter a reads quarter a^1  (p -> p xor 32)
        sc = (a ^ 1) * 32
        eng = nc.sync if a % 2 == 0 else nc.gpsimd
        eng.dma_start(out=sh[a * 32 : (a + 1) * 32, :], in_=raw[sc : sc + 32, :])
    tmp = rp.tile([128, 512], F32, name="rope_tmp", tag="rope_tmp")
    nc.vector.tensor_mul(tmp[:], raw[:], cos_sl)
    rot = rp.tile([128, 512], F32, name="rope_rot", tag="rope_rot")
    nc.gpsimd.tensor_mul(rot[:], sh[:], sgn_sl)
    fin = rp.tile([128, 512], BF16, name="rope_fin", tag="rope_fin")
    nc.vector.tensor_add(fin[:], tmp[:], rot[:])
    return fin


def _body(nc, tc, ctx, hid, wq, wk, wv, wo, cosd, sgnd, maskd, identd, onesd, onesrd, outd, qt_dram, dbg=None):
    # ---- constants (live whole body) ----
    cpool = ctx.enter_context(tc.tile_pool(name="const", bufs=1))
    ident_r = cpool.tile([128, 128], F32R, name="ident_r", tag="ident_r")
    nc.sync.dma_start(ident_r[:], identd[:].bitcast(F32R))

    with contextlib.ExitStack() as proj_ctx:
        tabp = proj_ctx.enter_context(tc.tile_pool(name="ropetab", bufs=1))
        cos128 = tabp.tile([128, S], F32, name="cos", tag="cos")
        nc.scalar.dma_start(cos128[:], cosd[:])
        sgn128 = tabp.tile([128, S], F32, name="sgn", tag="sgn")
        nc.scalar.dma_start(sgn128[:], sgnd[:])
        # shared weight-chunk pool: wv/wk/wq all stream [128, 8*512] chunks
        wbufp = proj_ctx.enter_context(tc.tile_pool(name="wbuf", bufs=4))
        # hT lives phases 1-4; va/kT live phases 2-5 (opened here, closed later)
        hTpool = proj_ctx.enter_context(tc.tile_pool(name="hTp", bufs=1))
        hT = [hTpool.tile([128, S], F32R, name=f"hT{c}", tag=f"hT{c}") for c in range(16)]

        attn_ctx = contextlib.ExitStack()
        vapool = attn_ctx.enter_context(tc.tile_pool(name="vap", bufs=1, side="right"))
        va = [
            vapool.tile([128, 8 * 65], BF16, name=f"va{s}", tag=f"va{s}")
            for s in range(8)
        ]
        kpool = attn_ctx.enter_context(tc.tile_pool(name="kTp", bufs=1, side="right"))
        kT = kpool.tile([128, 8 * S], BF16, name="kT", tag="kT")

        # ================= Phase 1: transpose hidden =================
        with tc.tile_pool(name="hidnat", bufs=5) as hp, tc.tile_pool(
            name="tpsum", bufs=6, space="PSUM"
        ) as tp:
            for half in range(2):
                hid_nat = []
                for tt in range(4):
                    t = half * 4 + tt
                    ht = hp.tile([128, H], F32R, name="hidnat", tag="hidnat")
                    nc.sync.dma_start(ht[:], hid[t * 128 : (t + 1) * 128, :].bitcast(F32R))
                    hid_nat.append(ht)
                for c in range(16):
                    ps = tp.tile([128, 512], F32R, name="tp", tag="tp")
                    for tt in range(4):
                        nc.tensor.transpose(
                            ps[:, tt * 128 : (tt + 1) * 128],
                            hid_nat[tt][:, c * 128 : (c + 1) * 128],
                            ident_r[:],
                        )
                    nc.scalar.copy(hT[c][:, half * 512 : (half + 1) * 512], ps[:])

        # ================= Phase 2: V projection (+ ones col) =========
        with tc.tile_pool(name="vpsum", bufs=8, space="PSUM") as vps:
            wv_t = []
            for c in range(2):
                wvm = wbufp.tile([128, 8 * 512], F32R, name="wvm", tag="wchunk")
                nc.sync.dma_start(
                    wvm.rearrange("p (t f) -> p t f", t=8),
                    wv.rearrange("(t p) f -> p t f", p=128)[:, c * 8 : c * 8 + 8].bitcast(F32R),
                )
                wv_t += [wvm[:, h * 512 : (h + 1) * 512] for h in range(8)]
            for st in range(8):
                ps = vps.tile([128, 512], F32, name="vp", tag="vp")
                for h in range(16):
                    nc.tensor.matmul(
                        ps[:],
                        hT[h][:, st * 128 : (st + 1) * 128],
                        wv_t[h],
                        start=(h == 0),
                        stop=(h == 15),
                    )
                va3 = va[st].rearrange("p (g c) -> p g c", c=65)
                nc.scalar.copy(
                    va3[:, :, 0:64], ps[:].rearrange("p (g c) -> p g c", c=64)
                )
                nc.gpsimd.dma_start(out=va3[:, :, 64:65], in_=onesd[st * 128 : (st + 1) * 128, :].rearrange("p (g c) -> p g c", c=1))

        # ============ Phase 3: K projection + RoPE + slot duplication ==
        with tc.tile_pool(
            name="kpsum", bufs=8, space="PSUM"
        ) as kps, tc.tile_pool(name="krope", bufs=4) as krp:
            wk_t = []
            for c in range(2):
                wkm = wbufp.tile([128, 8 * 512], F32R, name="wkm", tag="wchunk")
                nc.sync.dma_start(
                    wkm.rearrange("p (t f) -> p t f", t=8),
                    wk.rearrange("(t p) f -> p t f", p=128)[:, c * 8 : c * 8 + 8].bitcast(F32R),
                )
                wk_t += [wkm[:, h * 512 : (h + 1) * 512] for h in range(8)]
            for ft in range(4):
                for ih in range(2):
                    ps = kps.tile([128, 512], F32, name="kp", tag="kp")
                    for h in range(16):
                        nc.tensor.matmul(
                            ps[:],
                            wk_t[h][:, ft * 128 : (ft + 1) * 128],
                            hT[h][:, ih * 512 : (ih + 1) * 512],
                            start=(h == 0),
                            stop=(h == 15),
                        )
                    sl = slice(ih * 512, (ih + 1) * 512)
                    kfin = _rope(nc, krp, ps, cos128[:, sl], sgn128[:, sl])
                    b0, b1 = 2 * ft, 2 * ft + 1
                    o0 = b0 * S + ih * 512
                    o1 = b1 * S + ih * 512
                    nc.scalar.dma_start(kT[0:64, o0 : o0 + 512], kfin[0:64, :])
                    nc.gpsimd.dma_start(out=kT[64:128, o0 : o0 + 512], in_=kfin[0:64, :])
                    nc.scalar.dma_start(kT[64:128, o1 : o1 + 512], kfin[64:128, :])
                    nc.gpsimd.dma_start(out=kT[0:64, o1 : o1 + 512], in_=kfin[64:128, :])

        # ========= Phase 4: Q projection + RoPE -> DRAM spill ==========
        with tc.tile_pool(
            name="qpsum", bufs=8, space="PSUM"
        ) as qps, tc.tile_pool(name="qrope", bufs=4) as qrp:
            for wh in range(4):
                wq_t = []
                for c in range(2):
                    wqm = wbufp.tile([128, 8 * 512], F32R, name="wqm", tag="wchunk")
                    nc.sync.dma_start(
                        wqm.rearrange("p (t f) -> p t f", t=8),
                        wq.rearrange("(t p) f -> p t f", p=128)[
                            :, c * 8 : c * 8 + 8, wh * 512 : (wh + 1) * 512
                        ].bitcast(F32R),
                    )
                    wq_t += [wqm[:, h * 512 : (h + 1) * 512] for h in range(8)]
                for ftl in range(4):
                    ft = wh * 4 + ftl
                    for ih in range(2):
                        ps = qps.tile([128, 512], F32, name="qp", tag="qp")
                        for h in range(16):
                            nc.tensor.matmul(
                                ps[:],
                                wq_t[h][:, ftl * 128 : (ftl + 1) * 128],
                                hT[h][:, ih * 512 : (ih + 1) * 512],
                                start=(h == 0),
                                stop=(h == 15),
                            )
                        sl = slice(ih * 512, (ih + 1) * 512)
                        qfin = _rope(nc, qrp, ps, cos128[:, sl], sgn128[:, sl])
                        off = ft * S + ih * 512
                        nc.scalar.dma_start(qt_dram[:, off : off + 512], qfin[:])

    # hT freed here; attn_ctx (va, kT) still open
    # ================= Phase 5: attention =================
    mkp = ctx.enter_context(tc.tile_pool(name="masks", bufs=1))
    mask_b = mkp.tile([128, 128], BF16, name="mask_b", tag="mask_b")
    nc.gpsimd.dma_start(out=mask_b[:], in_=maskd[0:128, 0:128])
    ident_b = mkp.tile([128, 128], BF16, name="ident_b", tag="ident_b")
    nc.gpsimd.dma_start(out=ident_b[:], in_=identd[:])
    ones_r = mkp.tile([1, 64], BF16, name="ones_r", tag="ones_r")
    nc.gpsimd.dma_start(out=ones_r[:], in_=onesrd[:])
    wo0 = mkp.tile([128, 8 * 512], F32R, name="wo0", tag="wo0")
    nc.sync.dma_start(
        wo0.rearrange("p (t f) -> p t f", t=8),
        wo.rearrange("(t p) f -> p t f", p=128)[:, 0:8, 0:512].bitcast(F32R),
    )
    apool = ctx.enter_context(tc.tile_pool(name="attTp", bufs=1))
    attT = apool.tile([128, 16 * S], F32R, name="attT", tag="attT")

    with tc.tile_pool(name="qst", bufs=4) as qsp, tc.tile_pool(
        name="scpsum", bufs=2, space="PSUM"
    ) as scp, tc.tile_pool(name="pvpsum", bufs=1, space="PSUM") as pvp, tc.tile_pool(
        name="expT", bufs=5
    ) as exp_p, tc.tile_pool(name="pvsb", bufs=4) as pvsbp, tc.tile_pool(
        name="dbps", bufs=1, space="PSUM"
    ) as dbp, tc.tile_pool(name="rrowp", bufs=3) as rrp:
        for bq in range(16):
            qs = qsp.tile([128, 1024], BF16, name="qs", tag="qs")
            nc.sync.dma_start(qs[:], qt_dram[:, bq * S : bq * S + 1024])
            for hs in range(2):
                h = 2 * bq + hs
                g = h // 4
                slot = 64 * hs
                pv = pvp.tile([65, 1024], F32, name="pv", tag="pv")
                pvs = pvsbp.tile([65, 1024], F32R, name="pvs", tag="pvs")
                for jt in range(8):
                    lo = jt * 128
                    sc = scp.tile([128, 1024], F32, name="sc", tag="sc")
                    kap = kT[slot : slot + 64, g * S + lo : g * S + lo + 128]
                    qap = qs[slot : slot + 64, :]
                    vab = va[jt].rearrange("p (g c) -> p g c", c=65)[:, g, :]
                    if jt < 4:
                        nc.tensor.matmul(
                            sc[:, lo:512], kap, qap[:, lo:512],
                            start=True, stop=False, skip_group_check=True,
                        )
                        nc.tensor.matmul(
                            sc[:, 512:1024], kap, qap[:, 512:1024],
                            start=True, stop=True, skip_group_check=True,
                        )
                        nc.tensor.matmul(
                            sc[:, lo : lo + 128], ident_b[:], mask_b[:],
                            start=False, stop=True, skip_group_check=True,
                        )
                        ex = exp_p.tile([128, 1024], BF16, name="ex", tag="ex")
                        nc.scalar.activation(ex[:, lo:1024], sc[:, lo:1024], AF.Exp, scale=0.125)
                        nc.tensor.matmul(
                            pv[:, lo:512], vab, ex[:, lo:512],
                            start=(jt == 0), stop=(jt == 3), skip_group_check=True,
                        )
                        nc.tensor.matmul(
                            pv[:, 512:1024], vab, ex[:, 512:1024],
                            start=(jt == 0), stop=(jt == 7), skip_group_check=True,
                        )
                    else:
                        nc.tensor.matmul(
                            sc[:, lo:1024], kap, qap[:, lo:1024],
                            start=True, stop=False, skip_group_check=True,
                        )
                        nc.tensor.matmul(
                            sc[:, lo : lo + 128], ident_b[:], mask_b[:],
                            start=False, stop=True, skip_group_check=True,
                        )
                        ex = exp_p.tile([128, 1024], BF16, name="ex", tag="ex")
                        nc.scalar.activation(ex[:, lo:1024], sc[:, lo:1024], AF.Exp, scale=0.125)
                        nc.tensor.matmul(
                            pv[:, lo:1024], vab, ex[:, lo:1024],
                            start=False, stop=(jt == 7), skip_group_check=True,
                        )
                nc.vector.tensor_copy(pvs[:], pv[:])
                dstg = rrp.tile([1, 1024], F32, name="dstg", tag="dstg")
                nc.sync.dma_start(dstg[:], pvs[64:65, :].bitcast(F32))
                rrow = rrp.tile([1, 1024], F32, name="rrow", tag="rrow")
                nc.vector.reciprocal_approx_fast(rrow[:], dstg[:])
                rrb = rrp.tile([1, 1024], BF16, name="rrb", tag="rrb")
                nc.gpsimd.tensor_copy(rrb[:], rrow[:])
                db = dbp.tile([64, 1024], F32, name="db", tag="db")
                for ih in range(2):
                    nc.tensor.matmul(
                        db[:, ih * 512 : ih * 512 + 512],
                        ones_r[:],
                        rrb[0:1, ih * 512 : ih * 512 + 512],
                        start=True,
                        stop=True,
                    )
                pvn = pvsbp.tile([64, 1024], F32R, name="pvn", tag="pvn")
                nc.vector.tensor_mul(pvn[:], pvs[0:64, :], db[:].bitcast(F32R))
                nc.sync.dma_start(
                    attT[slot : slot + 64, bq * S : bq * S + 1024], pvn[:]
                )
                if dbg is not None and h == 0:
                    nc.sync.dma_start(dbg["pvs0"][:], pvs[:].bitcast(F32))
                    dbsb = pvsbp.tile([64, 1024], F32, name="dbsb", tag="dbsb")
                    nc.vector.tensor_copy(dbsb[:], db[:])
                    nc.sync.dma_start(dbg["db0"][:], dbsb[:])

    attn_ctx.close()  # free va, kT

    if dbg is not None:
        nc.sync.dma_start(dbg["attT"][:], attT[:].bitcast(F32))

    # ================= O projection ================
    with tc.tile_pool(name="wo", bufs=4) as wop, tc.tile_pool(
        name="opsum", bufs=4, space="PSUM"
    ) as ops, tc.tile_pool(name="osb", bufs=6) as osbp:
        for ho in range(4):
            woc = []
            for c in range(2):
                if ho == 0 and c == 0:
                    woc += [wo0[:, ft * 512 : (ft + 1) * 512] for ft in range(8)]
                    continue
                wom = wop.tile([128, 8 * 512], F32R, name="wom", tag="wom")
                nc.sync.dma_start(
                    wom.rearrange("p (t f) -> p t f", t=8),
                    wo.rearrange("(t p) f -> p t f", p=128)[
                        :, c * 8 : c * 8 + 8, ho * 512 : (ho + 1) * 512
                    ].bitcast(F32R),
                )
                woc += [wom[:, ft * 512 : (ft + 1) * 512] for ft in range(8)]
            for st in range(8):
                ps = ops.tile([128, 512], F32, name="op", tag="op")
                for ft in range(16):
                    nc.tensor.matmul(
                        ps[:],
                        attT[:, ft * S + st * 128 : ft * S + st * 128 + 128],
                        woc[ft],
                        start=(ft == 0),
                        stop=(ft == 15),
                    )
                ob = osbp.tile([128, 512], F32, name="ob", tag="ob")
                nc.scalar.copy(ob[:], ps[:])
                nc.gpsimd.dma_start(
                    out=outd[st * 128 : (st + 1) * 128, ho * 512 : (ho + 1) * 512],
                    in_=ob[:],
                )


def _build(niter=1, debug=False):
    nc = bacc.Bacc(None, target_bir_lowering=False)
    hid = nc.declare_dram_parameter("hidden_states", [S, H], F32, isOutput=False)
    wq = nc.declare_dram_parameter("Wq", [H, NQ * D], F32, isOutput=False)
    wk = nc.declare_dram_parameter("Wk", [H, NKV * D], F32, isOutput=False)
    wv = nc.declare_dram_parameter("Wv", [H, NKV * D], F32, isOutput=False)
    wo = nc.declare_dram_parameter("Wo", [NQ * D, H], F32, isOutput=False)
    cosd = nc.declare_dram_parameter("rope_cos", [128, S], F32, isOutput=False)
    sgnd = nc.declare_dram_parameter("rope_sgnsin", [128, S], F32, isOutput=False)
    maskd = nc.declare_dram_parameter("causal_masks", [512, 512], F32, isOutput=False)
    identd = nc.declare_dram_parameter("ident", [128, 128], F32, isOutput=False)
    onesd = nc.declare_dram_parameter("ones_col", [S, 8], F32, isOutput=False)
    onesrd = nc.declare_dram_parameter("ones_row", [1, 64], F32, isOutput=False)
    outd = nc.declare_dram_parameter("out", [S, H], F32, isOutput=True)
    dbg = None
    if debug:
        dbg = {
            "attT": nc.declare_dram_parameter("dbg_attT", [128, 16 * S], F32, isOutput=True),
            "pvs0": nc.declare_dram_parameter("dbg_pvs0", [65, 1024], F32, isOutput=True),
            "db0": nc.declare_dram_parameter("dbg_db0", [64, 1024], F32, isOutput=True),
        }
    qt_dram = nc.dram_tensor("qt_spill", [128, 16 * S], BF16)

    with tile.TileContext(nc) as tc:
        for _ in range(niter):
            with contextlib.ExitStack() as ctx:
                _body(
                    nc, tc, ctx, hid, wq, wk, wv, wo, cosd, sgnd, maskd, identd,
                    onesd, onesrd, outd, qt_dram, dbg,
                )
    nc.compile()
    return nc


_CACHE = {}


def _get_nc(niter=1):
    if niter not in _CACHE:
        _CACHE[niter] = _build(niter)
    return _CACHE[niter]


def _in_maps(inputs):
    cos128, sgn128, masks, ident = _tables()
    hidden = np.ascontiguousarray(inputs["hidden_states"], dtype=np.float32)
    base = {
        "Wq": np.ascontiguousarray(inputs["Wq"], dtype=np.float32),
        "Wk": np.ascontiguousarray(inputs["Wk"], dtype=np.float32),
        "Wv": np.ascontiguousarray(inputs["Wv"], dtype=np.float32),
        "Wo": np.ascontiguousarray(inputs["Wo"], dtype=np.float32),
        "rope_cos": cos128,
        "rope_sgnsin": sgn128,
        "causal_masks": masks,
        "ident": ident,
        "ones_col": np.ones((S, 8), np.float32),
        "ones_row": np.ones((1, 64), np.float32),
    }
    return [dict(base, hidden_states=hidden[i]) for i in range(B)]


def kernel(**inputs):
    nc = _get_nc(1)
    res = run_bass_kernel_spmd(nc, _in_maps(inputs), core_ids=list(range(8)))
    return np.stack([res.results[i]["out"] for i in range(B)]).astype(np.float32)
